# revision 1
# baseline (speedup 1.0000x reference)
"""DTCWT 3-level inverse on 8 Trainium2 NeuronCores.

Every filtering stage is a banded matmul on the tensor engine in fp32r
(tf32-like, ~1.4e-4 rel err per stage, 1 col/cycle at N>=256).

All stages use "data as lhsT" mode: matmul(out, lhsT=data[K=h, M=w],
rhs=mat[K=h, N=h_out]) contracts over the partition dim of the data and
yields the filtered image TRANSPOSED ([w, h_out]); column and row stages
then alternate orientation naturally with zero explicit transposes.

The c2q band construction is folded into the matrices: the interleaved-row
structure becomes even/odd-row submatrix stacks applied to partition-stacked
complex pair tiles, and the interleaved-column structure becomes even/odd
polyphase outputs, produced side by side by a column-concatenated rhs
([E | O]) since fp32r matmuls must write PSUM at partition base 0.

Sharding: pure data parallel over batch N (8 cores x 16 channels each).
"""
import sys

for _p in ('/opt/trn_rl_repo',):
    if _p not in sys.path:
        sys.path.append(_p)

import numpy as np
import concourse.bass as bass
import concourse.mybir as mybir
from concourse.tile import TileContext
from concourse.bass_utils import run_bass_kernel_spmd

SQRT_HALF = 0.7071067811865476
N_CORES = 8
IMGS_PER_CORE = 16
F32 = mybir.dt.float32
F32R = mybir.dt.float32r


# ---------------------------------------------------------------------------
# Host-side matrix construction (numpy, float64)
# ---------------------------------------------------------------------------
def _conv_rows_valid(x, h):
    hr = h[::-1]
    taps = h.shape[0]
    n = x.shape[-2] - taps + 1
    out = hr[0] * x[..., 0:n, :]
    for k in range(1, taps):
        out = out + hr[k] * x[..., k:k + n, :]
    return out


def _pad_rows_symmetric(x, m):
    pad = [(0, 0)] * (x.ndim - 2) + [(m, m), (0, 0)]
    return np.pad(x, pad, mode='symmetric')


def _colfilter(x, h):
    return _conv_rows_valid(_pad_rows_symmetric(x, h.shape[0] // 2), h)


def _colifilt(x, ha, hb, highpass):
    m = ha.shape[0]
    m2 = m // 2
    r = x.shape[-2]
    xp = _pad_rows_symmetric(x, m2)
    xe = xp[..., 1:r + m - 2:2, :]
    xo = xp[..., 2:r + m - 1:2, :]
    xa, xb = (xe, xo) if highpass else (xo, xe)
    hao, hae = ha[0::2], ha[1::2]
    hbo, hbe = hb[0::2], hb[1::2]
    y0 = _conv_rows_valid(xb, hao)
    y1 = _conv_rows_valid(xa, hbo)
    y2 = _conv_rows_valid(xb, hae)
    y3 = _conv_rows_valid(xa, hbe)
    y = np.stack([y0, y1, y2, y3], axis=-2)
    return y.reshape(y.shape[:-3] + (2 * r, y.shape[-1]))


def _op_matrix(op, n):
    """M[h_in, h_out] with out[h_out, w] = sum_h M[h, h_out] x[h, w]."""
    return np.ascontiguousarray(op(np.eye(n, dtype=np.float64)).T)


def build_matrices(g0o, g1o, g0a, g0b, g1a, g1b):
    """All device matrices as {name: fp32 ndarray}."""
    g0o = np.asarray(g0o, np.float64)
    g1o = np.asarray(g1o, np.float64)
    g0a = np.asarray(g0a, np.float64)
    g0b = np.asarray(g0b, np.float64)
    g1a = np.asarray(g1a, np.float64)
    g1b = np.asarray(g1b, np.float64)
    s = SQRT_HALF
    hs, vs = np.hstack, np.vstack
    out = {}

    def upsample_level(R, tag):
        Mlo = _op_matrix(lambda x: _colifilt(x, g0b, g0a, False), R)  # [R, 2R]
        Mhi = _op_matrix(lambda x: _colifilt(x, g1b, g1a, True), R)
        Me_h, Mo_h = s * Mhi[0::2], s * Mhi[1::2]                     # [R/2, 2R]
        Me_l, Mo_l = s * Mlo[0::2], s * Mlo[1::2]
        out[f'M{tag}_lo'] = Mlo
        # pair-stacked [w1; w2] col rhs, e|o column-concatenated
        #   e: w1r*Me + w2r*Me + w1i*Mo - w2i*Mo
        #   o: -w1r*Mo + w2r*Mo + w1i*Me + w2i*Me
        out[f'L{tag}_hi_R'] = hs([vs([Me_h, Me_h]), vs([-Mo_h, Mo_h])])
        out[f'L{tag}_hi_I'] = hs([vs([Mo_h, -Mo_h]), vs([Me_h, Me_h])])
        out[f'L{tag}_lo_R'] = hs([vs([Me_l, Me_l]), vs([-Mo_l, Mo_l])])
        out[f'L{tag}_lo_I'] = hs([vs([Mo_l, -Mo_l]), vs([Me_l, Me_l])])
        # row stage (polyphase-column recombination)
        out[f'Be{tag}_lo'], out[f'Bo{tag}_lo'] = Mlo[0::2], Mlo[1::2]
        out[f'Be{tag}_hi'], out[f'Bo{tag}_hi'] = Mhi[0::2], Mhi[1::2]
        return Me_h, Mo_h, Me_l, Mo_l

    upsample_level(64, '3')
    upsample_level(128, '2')
    # L3 quad stacks: [hl pair (lo mats); hh pair (hi mats)], K=128
    out['L3_q_R'] = vs([out['L3_lo_R'], out['L3_hi_R']])
    out['L3_q_I'] = vs([out['L3_lo_I'], out['L3_hi_I']])
    del out['L3_lo_R'], out['L3_lo_I']  # only used inside the quad at L3
    # K-stacked row-stage rhs (lhsT pieces partition-stacked via shift DMAs)
    out['R3_A'] = vs([out['M3_lo'], out['Be3_lo'], out['Bo3_lo']])
    out['R3_B'] = vs([out['Be3_hi'], out['Bo3_hi']])
    out['R2_E'] = vs([out['Be2_lo'], out['Be2_hi']])
    out['R2_O'] = vs([out['Bo2_lo'], out['Bo2_hi']])
    for k in ('Be3_lo', 'Bo3_lo', 'Be3_hi', 'Bo3_hi',
              'Be2_lo', 'Bo2_lo', 'Be2_hi', 'Bo2_hi'):
        del out[k]

    # L1 (colfilter, size-preserving, n=256)
    A_lo = _op_matrix(lambda x: _colfilter(x, g0o), 256)              # [256, 256]
    A_hi = _op_matrix(lambda x: _colfilter(x, g1o), 256)
    out['Alo_a'], out['Alo_b'] = A_lo[0:128], A_lo[128:256]
    for x, A in (('hi', A_hi), ('lo', A_lo)):
        Me, Mo = s * A[0::2], s * A[1::2]                             # [128, 256]
        out[f'L1{x}_w1r'] = hs([Me, -Mo])
        out[f'L1{x}_w2r'] = hs([Me, Mo])
        out[f'L1{x}_w1i'] = hs([Mo, Me])
        out[f'L1{x}_w2i'] = hs([-Mo, Me])
    out['Be1_lo'], out['Bo1_lo'] = A_lo[0::2], A_lo[1::2]
    out['Be1_hi'], out['Bo1_hi'] = A_hi[0::2], A_hi[1::2]
    return {k: np.ascontiguousarray(v, np.float32) for k, v in out.items()}


MAT_SHAPES = {
    'M3_lo': (64, 128),
    'L3_hi_R': (64, 256), 'L3_hi_I': (64, 256),
    'L3_q_R': (128, 256), 'L3_q_I': (128, 256),
    'R3_A': (128, 128), 'R3_B': (64, 128),
    'M2_lo': (128, 256),
    'L2_hi_R': (128, 512), 'L2_hi_I': (128, 512),
    'L2_lo_R': (128, 512), 'L2_lo_I': (128, 512),
    'R2_E': (128, 256), 'R2_O': (128, 256),
    'Alo_a': (128, 256), 'Alo_b': (128, 256),
    'L1hi_w1r': (128, 512), 'L1hi_w2r': (128, 512),
    'L1hi_w1i': (128, 512), 'L1hi_w2i': (128, 512),
    'L1lo_w1r': (128, 512), 'L1lo_w2r': (128, 512),
    'L1lo_w1i': (128, 512), 'L1lo_w2i': (128, 512),
    'Be1_lo': (128, 256), 'Bo1_lo': (128, 256),
    'Be1_hi': (128, 256), 'Bo1_hi': (128, 256),
}


# ---------------------------------------------------------------------------
# Bass kernel
# ---------------------------------------------------------------------------
def split_excess_waits(nc, max_waits=1):
    """walrus CTRL codegen allows only one sem wait per instruction; move
    excess waits onto NoOps inserted just before the offending instruction."""
    ctr = 0
    for fn in nc.m.functions:
        for bb in fn.blocks:
            new_list = []
            for inst in bb.instructions:
                si = inst.sync_info
                if si is not None and si.on_wait and len(si.on_wait) > max_waits:
                    waits = list(si.on_wait)
                    keep, extra = waits[:max_waits], waits[max_waits:]
                    for i in range(0, len(extra), max_waits):
                        nop = mybir.InstNoOp(
                            name=f"wait_split_{ctr}", ins=[], outs=[])
                        ctr += 1
                        nop.engine = inst.engine
                        nop.sync_info = mybir.SyncInfo(
                            on_wait=extra[i:i + max_waits], on_update=[])
                        nc.register_instruction(nop)
                        new_list.append(nop)
                    inst.sync_info = mybir.SyncInfo(
                        on_wait=keep,
                        on_update=list(si.on_update) if si.on_update else [])
                new_list.append(inst)
            bb.instructions[:] = new_list
    return ctr


def build_nc():
    nc = bass.Bass()
    yl_d = nc.dram_tensor("yl", [IMGS_PER_CORE, 64, 64], F32R,
                          kind="ExternalInput")
    yh2_d = nc.dram_tensor("yh2", [IMGS_PER_CORE, 6, 32, 32, 2], F32R,
                           kind="ExternalInput")
    yh1_d = nc.dram_tensor("yh1", [IMGS_PER_CORE, 6, 64, 64, 2], F32R,
                           kind="ExternalInput")
    yh0_d = nc.dram_tensor("yh0", [IMGS_PER_CORE, 6, 128, 128, 2], F32R,
                           kind="ExternalInput")
    out_d = nc.dram_tensor("out", [IMGS_PER_CORE, 256, 256], F32,
                           kind="ExternalOutput")
    mat_d = {k: nc.dram_tensor(k, list(shp), F32R, kind="ExternalInput")
             for k, shp in MAT_SHAPES.items()}

    def flat2(ap):
        return ap.rearrange("a b c -> a (b c)")

    with TileContext(nc) as tc:
        with tc.tile_pool(name="mats", bufs=1) as matpool, \
             tc.tile_pool(name="data", bufs=4) as datapool, \
             tc.tile_pool(name="mid", bufs=4) as midpool, \
             tc.tile_pool(name="outp", bufs=4) as outpool, \
             tc.tile_pool(name="ps3", bufs=1, space="PSUM") as ps3pool, \
             tc.tile_pool(name="ps2c", bufs=1, space="PSUM") as ps2cpool, \
             tc.tile_pool(name="ps2r", bufs=1, space="PSUM") as ps2rpool, \
             tc.tile_pool(name="ps1c", bufs=1, space="PSUM") as ps1cpool, \
             tc.tile_pool(name="ps1r", bufs=1, space="PSUM") as ps1rpool:

            # L3/L2 matrices first so image-0 matmuls can start early;
            # spread the loads across the three DMA-capable sequencers.
            mats = {}
            _order = sorted(MAT_SHAPES, key=lambda k: k.startswith(
                ('Alo', 'L1', 'Be1', 'Bo1')))
            for i, k in enumerate(_order):
                t = matpool.tile(list(MAT_SHAPES[k]), F32R, tag=f"mat_{k}")
                eng = (nc.sync, nc.scalar)[i % 2]
                eng.dma_start(out=t[:], in_=mat_d[k][:])
                mats[k] = t

            def mm(out_ap, lhsT, rhs_name, start, stop):
                nc.tensor.matmul(out_ap, lhsT, mats[rhs_name][:],
                                 start=start, stop=stop)

            for img in range(IMGS_PER_CORE):
                # =======================================================
                # Level 3: z3 [64,64] + yh2 bands -> z2 [128,128]
                # =======================================================
                z3 = datapool.tile([64, 64], F32R, tag="z3")
                nc.gpsimd.dma_start(out=z3[:], in_=yl_d[img])
                lh3 = datapool.tile([64, 64], F32R, tag="lh3")
                nc.gpsimd.dma_start(out=lh3[0:32, :], in_=flat2(yh2_d[img, 0]))
                nc.gpsimd.dma_start(out=lh3[32:64, :], in_=flat2(yh2_d[img, 5]))
                q3 = datapool.tile([128, 64], F32R, tag="q3")
                nc.gpsimd.dma_start(out=q3[0:32, :], in_=flat2(yh2_d[img, 2]))
                nc.gpsimd.dma_start(out=q3[32:64, :], in_=flat2(yh2_d[img, 3]))
                nc.gpsimd.dma_start(out=q3[64:96, :], in_=flat2(yh2_d[img, 1]))
                nc.gpsimd.dma_start(out=q3[96:128, :], in_=flat2(yh2_d[img, 4]))

                # p3 [128, 1024] (2 banks): y1z [64: 0-128), y1b [32: 128-384),
                #   z2 [128: 384-512), y2b [32: 512-768)
                p3 = ps3pool.tile([128, 1024], F32, tag="p3")
                mm(p3[0:64, 0:128], z3[:], 'M3_lo', True, True)
                lh3R, lh3I = lh3[:, 0::2], lh3[:, 1::2]
                mm(p3[0:32, 128:384], lh3R, 'L3_hi_R', True, False)
                mm(p3[0:32, 128:384], lh3I, 'L3_hi_I', False, True)
                q3R, q3I = q3[:, 0::2], q3[:, 1::2]
                mm(p3[0:32, 512:768], q3R, 'L3_q_R', True, False)
                mm(p3[0:32, 512:768], q3I, 'L3_q_I', False, True)

                # K-stack the row-stage pieces: rowA = [y1z; y1b_e; y1b_o],
                # rowB = [y2b_e; y2b_o] (partition shifts via SBUF-SBUF DMA)
                rowA = midpool.tile([128, 128], F32R, tag="rowA3")
                nc.scalar.copy(rowA[0:64, :], p3[0:64, 0:128])
                y1b_s = midpool.tile([32, 256], F32R, tag="y1b3")
                nc.vector.tensor_copy(out=y1b_s[:], in_=p3[0:32, 128:384])
                y2b_s = midpool.tile([32, 256], F32R, tag="y2b3")
                nc.vector.tensor_copy(out=y2b_s[:], in_=p3[0:32, 512:768])
                nc.scalar.dma_start(out=rowA[64:96, :], in_=y1b_s[:, 0:128])
                nc.scalar.dma_start(out=rowA[96:128, :], in_=y1b_s[:, 128:256])
                rowB = midpool.tile([64, 128], F32R, tag="rowB3")
                nc.vector.tensor_copy(out=rowB[0:32, :], in_=y2b_s[:, 0:128])
                nc.scalar.dma_start(out=rowB[32:64, :], in_=y2b_s[:, 128:256])

                z2_p = p3[0:128, 384:512]
                mm(z2_p, rowA[:], 'R3_A', True, False)
                mm(z2_p, rowB[:], 'R3_B', False, True)
                z2_s = midpool.tile([128, 128], F32R, tag="z2")
                nc.scalar.copy(z2_s[:], z2_p)

                # =======================================================
                # Level 2: z2 [128,128] + yh1 bands -> z1 [256,256]
                # =======================================================
                lh2 = datapool.tile([128, 128], F32R, tag="lh2")
                nc.gpsimd.dma_start(out=lh2[0:64, :], in_=flat2(yh1_d[img, 0]))
                nc.gpsimd.dma_start(out=lh2[64:128, :],
                                    in_=flat2(yh1_d[img, 5]))
                hl2 = datapool.tile([128, 128], F32R, tag="hl2")
                nc.gpsimd.dma_start(out=hl2[0:64, :], in_=flat2(yh1_d[img, 2]))
                nc.gpsimd.dma_start(out=hl2[64:128, :],
                                    in_=flat2(yh1_d[img, 3]))
                hh2 = datapool.tile([128, 128], F32R, tag="hh2")
                nc.gpsimd.dma_start(out=hh2[0:64, :], in_=flat2(yh1_d[img, 1]))
                nc.gpsimd.dma_start(out=hh2[64:128, :],
                                    in_=flat2(yh1_d[img, 4]))

                # phase A: p2 [128, 1024]: y1zT [0:256), b1 [512:1024)
                p2a = ps2cpool.tile([128, 1024], F32, tag="p2")
                mm(p2a[:, 0:256], z2_s[:], 'M2_lo', True, True)
                lh2R, lh2I = lh2[:, 0::2], lh2[:, 1::2]
                mm(p2a[0:64, 512:1024], lh2R, 'L2_hi_R', True, False)
                mm(p2a[0:64, 512:1024], lh2I, 'L2_hi_I', False, True)
                y1zT_s = midpool.tile([128, 256], F32R, tag="y1zT2")
                nc.scalar.copy(y1zT_s[:], p2a[:, 0:256])
                # row2 = [b1; b2] K-stacked ([e | o] along columns)
                row2 = midpool.tile([128, 512], F32R, tag="row2")
                nc.vector.tensor_copy(out=row2[0:64, :],
                                      in_=p2a[0:64, 512:1024])

                # phase B (same slot): b2 [0:512)
                p2b = ps2cpool.tile([128, 1024], F32, tag="p2")
                hl2R, hl2I = hl2[:, 0::2], hl2[:, 1::2]
                hh2R, hh2I = hh2[:, 0::2], hh2[:, 1::2]
                mm(p2b[0:64, 0:512], hl2R, 'L2_lo_R', True, False)
                mm(p2b[0:64, 0:512], hl2I, 'L2_lo_I', False, False)
                mm(p2b[0:64, 0:512], hh2R, 'L2_hi_R', False, False)
                mm(p2b[0:64, 0:512], hh2I, 'L2_hi_I', False, True)
                b2_s = midpool.tile([64, 512], F32R, tag="b2_2")
                nc.vector.tensor_copy(out=b2_s[:], in_=p2b[0:64, 0:512])
                nc.scalar.dma_start(out=row2[64:128, :], in_=b2_s[:])

                # row stage -> z1 [256,256] as [128, 512] (cols 0:256 = rows
                # 0:128, cols 256:512 = rows 128:256)
                p2r = ps2rpool.tile([128, 512], F32, tag="p2r")
                z1_s = midpool.tile([128, 512], F32R, tag="z1")
                for m in range(2):
                    zc = p2r[:, m * 256:(m + 1) * 256]
                    msl = slice(m * 128, (m + 1) * 128)
                    mm(zc, y1zT_s[:, msl], 'M2_lo', True, False)
                    mm(zc, row2[:, msl], 'R2_E', False, False)
                    mm(zc, row2[:, 256 + m * 128:256 + (m + 1) * 128],
                       'R2_O', False, True)
                    if m == 0:
                        nc.scalar.copy(z1_s[:, 0:256], zc)
                    else:
                        nc.vector.tensor_copy(out=z1_s[:, 256:512], in_=zc)

                # =======================================================
                # Level 1: z1 [256,256] + yh0 bands [128,128] -> out
                # =======================================================
                o_t = {}
                for o in range(6):
                    t = datapool.tile([128, 256], F32R, tag=f"yh0_o{o}")
                    eng = nc.sync if o % 2 == 0 else nc.gpsimd
                    eng.dma_start(out=t[:], in_=flat2(yh0_d[img, o]))
                    o_t[o] = t

                # phase A: p1 [128, 1024]: y1zT a|b [0:512), y1b [512:1024)
                p1a = ps1cpool.tile([128, 1024], F32, tag="p1")
                for m in range(2):
                    yz = p1a[:, m * 256:(m + 1) * 256]
                    mm(yz, z1_s[:, m * 128:m * 128 + 128], 'Alo_a',
                       True, False)
                    mm(yz, z1_s[:, 256 + m * 128:256 + m * 128 + 128],
                       'Alo_b', False, True)
                y1b_p = p1a[:, 512:1024]
                mm(y1b_p, o_t[0][:, 0::2], 'L1hi_w1r', True, False)
                mm(y1b_p, o_t[5][:, 0::2], 'L1hi_w2r', False, False)
                mm(y1b_p, o_t[0][:, 1::2], 'L1hi_w1i', False, False)
                mm(y1b_p, o_t[5][:, 1::2], 'L1hi_w2i', False, True)
                y1zT1_s = midpool.tile([128, 512], F32R, tag="y1zT1")
                nc.scalar.copy(y1zT1_s[:, 0:256], p1a[:, 0:256])
                nc.scalar.copy(y1zT1_s[:, 256:512], p1a[:, 256:512])
                y1b1_s = midpool.tile([128, 512], F32R, tag="y1b1")
                nc.vector.tensor_copy(out=y1b1_s[:], in_=y1b_p)

                # phase B (same slot): y2b e|o [0:512)
                p1b = ps1cpool.tile([128, 1024], F32, tag="p1")
                y2b_p = p1b[:, 0:512]
                mm(y2b_p, o_t[2][:, 0::2], 'L1lo_w1r', True, False)
                mm(y2b_p, o_t[3][:, 0::2], 'L1lo_w2r', False, False)
                mm(y2b_p, o_t[2][:, 1::2], 'L1lo_w1i', False, False)
                mm(y2b_p, o_t[3][:, 1::2], 'L1lo_w2i', False, False)
                mm(y2b_p, o_t[1][:, 0::2], 'L1hi_w1r', False, False)
                mm(y2b_p, o_t[4][:, 0::2], 'L1hi_w2r', False, False)
                mm(y2b_p, o_t[1][:, 1::2], 'L1hi_w1i', False, False)
                mm(y2b_p, o_t[4][:, 1::2], 'L1hi_w2i', False, True)
                y2b1_s = midpool.tile([128, 512], F32R, tag="y2b1")
                nc.vector.tensor_copy(out=y2b1_s[:], in_=y2b_p)

                # row stage -> out [256, 256] in two h-chunks
                p1r = ps1rpool.tile([128, 512], F32, tag="p1r")
                for m in range(2):
                    oc = p1r[:, m * 256:(m + 1) * 256]
                    msl = slice(m * 128, (m + 1) * 128)
                    osl = slice(256 + m * 128, 256 + (m + 1) * 128)
                    mm(oc, y1zT1_s[:, msl], 'Alo_a', True, False)
                    mm(oc, y1zT1_s[:, osl], 'Alo_b', False, False)
                    mm(oc, y1b1_s[:, msl], 'Be1_lo', False, False)
                    mm(oc, y1b1_s[:, osl], 'Bo1_lo', False, False)
                    mm(oc, y2b1_s[:, msl], 'Be1_hi', False, False)
                    mm(oc, y2b1_s[:, osl], 'Bo1_hi', False, True)
                    ot = outpool.tile([128, 256], F32, tag=f"out_m{m}")
                    if m == 0:
                        nc.scalar.copy(ot[:], oc)
                    else:
                        nc.vector.tensor_copy(out=ot[:], in_=oc)
                    nc.sync.dma_start(
                        out=out_d[img, m * 128:(m + 1) * 128, :], in_=ot[:])

    split_excess_waits(nc)
    return nc


# ---------------------------------------------------------------------------
# Entry point
# ---------------------------------------------------------------------------
_NC_CACHE = []
_LAST_RESULT = []  # last BassKernelResults (exec_time_ns when BASS_TRACE=1)


def _axon_reset():
    try:
        import ctypes
        lib = ctypes.CDLL('/opt/axon/libaxon_pjrt.so')
        lib.axon_reset.restype = ctypes.c_int64
        lib.axon_reset()
    except Exception:
        pass


def kernel(yl, yh0, yh1, yh2, g0o, g1o, g0a, g0b, g1a, g1b):
    yl = np.ascontiguousarray(np.asarray(yl, np.float32))
    yh0 = np.ascontiguousarray(np.asarray(yh0, np.float32))
    yh1 = np.ascontiguousarray(np.asarray(yh1, np.float32))
    yh2 = np.ascontiguousarray(np.asarray(yh2, np.float32))
    assert yl.shape == (8, 16, 64, 64)

    mats = build_matrices(g0o, g1o, g0a, g0b, g1a, g1b)
    if not _NC_CACHE:
        _NC_CACHE.append(build_nc())
    nc = _NC_CACHE[0]

    in_maps = []
    for core in range(N_CORES):
        m = {"yl": yl[core], "yh0": yh0[core],
             "yh1": yh1[core], "yh2": yh2[core]}
        for k in MAT_SHAPES:
            m[k] = mats[k]
        in_maps.append(m)

    try:
        res = run_bass_kernel_spmd(nc, in_maps, list(range(N_CORES)))
    except Exception as e:  # wedged exec unit: reset the axon device, retry
        if "UNAVAILABLE" not in str(e) and "unrecoverable" not in str(e):
            raise
        _axon_reset()
        res = run_bass_kernel_spmd(nc, in_maps, list(range(N_CORES)))
    _LAST_RESULT.clear()
    _LAST_RESULT.append(res)
    out = np.stack([res.results[i]["out"] for i in range(N_CORES)], axis=0)
    return np.ascontiguousarray(out.astype(np.float32))



# revision 8
# speedup vs baseline: 1.1320x; 1.1320x over previous
"""DTCWT 3-level inverse on 8 Trainium2 NeuronCores.

Every filtering stage is a banded matmul on the tensor engine in fp32r
(tf32-like, ~1.4e-4 rel err per stage, 1 col/cycle at N>=256).

All stages use "data as lhsT" mode: matmul(out, lhsT=data[K=h, M=w],
rhs=mat[K=h, N=h_out]) contracts over the partition dim of the data and
yields the filtered image TRANSPOSED ([w, h_out]); column and row stages
then alternate orientation naturally with zero explicit transposes.

The c2q band construction is folded into the matrices: the interleaved-row
structure becomes even/odd-row submatrix stacks applied to partition-stacked
complex pair tiles, and the interleaved-column structure becomes even/odd
polyphase outputs, produced side by side by a column-concatenated rhs
([E | O]) since fp32r matmuls must write PSUM at partition base 0.

Sharding: pure data parallel over batch N (8 cores x 16 channels each).
"""
import sys

for _p in ('/opt/trn_rl_repo',):
    if _p not in sys.path:
        sys.path.append(_p)

import numpy as np
import concourse.bass as bass
import concourse.mybir as mybir
from concourse.tile import TileContext
from concourse.bass_utils import run_bass_kernel_spmd

SQRT_HALF = 0.7071067811865476
N_CORES = 8
IMGS_PER_CORE = 16
F32 = mybir.dt.float32
F32R = mybir.dt.float32r
F16 = mybir.dt.float16


# ---------------------------------------------------------------------------
# Host-side matrix construction (numpy, float64)
# ---------------------------------------------------------------------------
def _conv_rows_valid(x, h):
    hr = h[::-1]
    taps = h.shape[0]
    n = x.shape[-2] - taps + 1
    out = hr[0] * x[..., 0:n, :]
    for k in range(1, taps):
        out = out + hr[k] * x[..., k:k + n, :]
    return out


def _pad_rows_symmetric(x, m):
    pad = [(0, 0)] * (x.ndim - 2) + [(m, m), (0, 0)]
    return np.pad(x, pad, mode='symmetric')


def _colfilter(x, h):
    return _conv_rows_valid(_pad_rows_symmetric(x, h.shape[0] // 2), h)


def _colifilt(x, ha, hb, highpass):
    m = ha.shape[0]
    m2 = m // 2
    r = x.shape[-2]
    xp = _pad_rows_symmetric(x, m2)
    xe = xp[..., 1:r + m - 2:2, :]
    xo = xp[..., 2:r + m - 1:2, :]
    xa, xb = (xe, xo) if highpass else (xo, xe)
    hao, hae = ha[0::2], ha[1::2]
    hbo, hbe = hb[0::2], hb[1::2]
    y0 = _conv_rows_valid(xb, hao)
    y1 = _conv_rows_valid(xa, hbo)
    y2 = _conv_rows_valid(xb, hae)
    y3 = _conv_rows_valid(xa, hbe)
    y = np.stack([y0, y1, y2, y3], axis=-2)
    return y.reshape(y.shape[:-3] + (2 * r, y.shape[-1]))


def _op_matrix(op, n):
    """M[h_in, h_out] with out[h_out, w] = sum_h M[h, h_out] x[h, w]."""
    return np.ascontiguousarray(op(np.eye(n, dtype=np.float64)).T)


def build_matrices(g0o, g1o, g0a, g0b, g1a, g1b):
    """All device matrices as {name: fp32 ndarray}."""
    g0o = np.asarray(g0o, np.float64)
    g1o = np.asarray(g1o, np.float64)
    g0a = np.asarray(g0a, np.float64)
    g0b = np.asarray(g0b, np.float64)
    g1a = np.asarray(g1a, np.float64)
    g1b = np.asarray(g1b, np.float64)
    s = SQRT_HALF
    hs, vs = np.hstack, np.vstack
    out = {}

    def upsample_level(R, tag):
        Mlo = _op_matrix(lambda x: _colifilt(x, g0b, g0a, False), R)  # [R, 2R]
        Mhi = _op_matrix(lambda x: _colifilt(x, g1b, g1a, True), R)
        Me_h, Mo_h = s * Mhi[0::2], s * Mhi[1::2]                     # [R/2, 2R]
        Me_l, Mo_l = s * Mlo[0::2], s * Mlo[1::2]
        out[f'M{tag}_lo'] = Mlo
        # pair-stacked [w1; w2] col rhs, e|o column-concatenated
        #   e: w1r*Me + w2r*Me + w1i*Mo - w2i*Mo
        #   o: -w1r*Mo + w2r*Mo + w1i*Me + w2i*Me
        out[f'L{tag}_hi_R'] = hs([vs([Me_h, Me_h]), vs([-Mo_h, Mo_h])])
        out[f'L{tag}_hi_I'] = hs([vs([Mo_h, -Mo_h]), vs([Me_h, Me_h])])
        out[f'L{tag}_lo_R'] = hs([vs([Me_l, Me_l]), vs([-Mo_l, Mo_l])])
        out[f'L{tag}_lo_I'] = hs([vs([Mo_l, -Mo_l]), vs([Me_l, Me_l])])
        # row stage (polyphase-column recombination)
        out[f'Be{tag}_lo'], out[f'Bo{tag}_lo'] = Mlo[0::2], Mlo[1::2]
        out[f'Be{tag}_hi'], out[f'Bo{tag}_hi'] = Mhi[0::2], Mhi[1::2]
        return Me_h, Mo_h, Me_l, Mo_l

    upsample_level(64, '3')
    upsample_level(128, '2')
    # L3 quad stacks: [hl pair (lo mats); hh pair (hi mats)], K=128
    out['L3_q_R'] = vs([out['L3_lo_R'], out['L3_hi_R']])
    out['L3_q_I'] = vs([out['L3_lo_I'], out['L3_hi_I']])
    del out['L3_lo_R'], out['L3_lo_I']  # only used inside the quad at L3
    # K-stacked row-stage rhs (lhsT pieces partition-stacked via shift DMAs)
    out['R3_A'] = vs([out['M3_lo'], out['Be3_lo'], out['Bo3_lo']])
    out['R3_B'] = vs([out['Be3_hi'], out['Bo3_hi']])
    out['R2_E'] = vs([out['Be2_lo'], out['Be2_hi']])
    out['R2_O'] = vs([out['Bo2_lo'], out['Bo2_hi']])
    for k in ('Be3_lo', 'Bo3_lo', 'Be3_hi', 'Bo3_hi',
              'Be2_lo', 'Bo2_lo', 'Be2_hi', 'Bo2_hi'):
        del out[k]

    # L1 (colfilter, size-preserving, n=256)
    A_lo = _op_matrix(lambda x: _colfilter(x, g0o), 256)              # [256, 256]
    A_hi = _op_matrix(lambda x: _colfilter(x, g1o), 256)
    out['Alo_a'], out['Alo_b'] = A_lo[0:128], A_lo[128:256]
    for x, A in (('hi', A_hi), ('lo', A_lo)):
        Me, Mo = s * A[0::2], s * A[1::2]                             # [128, 256]
        out[f'L1{x}_w1r'] = hs([Me, -Mo])
        out[f'L1{x}_w2r'] = hs([Me, Mo])
        out[f'L1{x}_w1i'] = hs([Mo, Me])
        out[f'L1{x}_w2i'] = hs([-Mo, Me])
    out['Be1_lo'], out['Bo1_lo'] = A_lo[0::2], A_lo[1::2]
    out['Be1_hi'], out['Bo1_hi'] = A_hi[0::2], A_hi[1::2]
    return {k: np.ascontiguousarray(v, np.float16) for k, v in out.items()}


MAT_SHAPES = {
    'M3_lo': (64, 128),
    'L3_hi_R': (64, 256), 'L3_hi_I': (64, 256),
    'L3_q_R': (128, 256), 'L3_q_I': (128, 256),
    'R3_A': (128, 128), 'R3_B': (64, 128),
    'M2_lo': (128, 256),
    'L2_hi_R': (128, 512), 'L2_hi_I': (128, 512),
    'L2_lo_R': (128, 512), 'L2_lo_I': (128, 512),
    'R2_E': (128, 256), 'R2_O': (128, 256),
    'Alo_a': (128, 256), 'Alo_b': (128, 256),
    'L1hi_w1r': (128, 512), 'L1hi_w2r': (128, 512),
    'L1hi_w1i': (128, 512), 'L1hi_w2i': (128, 512),
    'L1lo_w1r': (128, 512), 'L1lo_w2r': (128, 512),
    'L1lo_w1i': (128, 512), 'L1lo_w2i': (128, 512),
    'Be1_lo': (128, 256), 'Bo1_lo': (128, 256),
    'Be1_hi': (128, 256), 'Bo1_hi': (128, 256),
}


# ---------------------------------------------------------------------------
# Bass kernel
# ---------------------------------------------------------------------------
def split_excess_waits(nc, max_waits=1):
    """walrus CTRL codegen allows only one sem wait per instruction; move
    excess waits onto NoOps inserted just before the offending instruction."""
    ctr = 0
    for fn in nc.m.functions:
        for bb in fn.blocks:
            new_list = []
            for inst in bb.instructions:
                si = inst.sync_info
                if si is not None and si.on_wait and len(si.on_wait) > max_waits:
                    waits = list(si.on_wait)
                    keep, extra = waits[:max_waits], waits[max_waits:]
                    for i in range(0, len(extra), max_waits):
                        nop = mybir.InstNoOp(
                            name=f"wait_split_{ctr}", ins=[], outs=[])
                        ctr += 1
                        nop.engine = inst.engine
                        nop.sync_info = mybir.SyncInfo(
                            on_wait=extra[i:i + max_waits], on_update=[])
                        nc.register_instruction(nop)
                        new_list.append(nop)
                    inst.sync_info = mybir.SyncInfo(
                        on_wait=keep,
                        on_update=list(si.on_update) if si.on_update else [])
                new_list.append(inst)
            bb.instructions[:] = new_list
    return ctr


def build_nc():
    nc = bass.Bass()
    yl_d = nc.dram_tensor("yl", [IMGS_PER_CORE, 64, 64], F16,
                          kind="ExternalInput")
    yh2_d = nc.dram_tensor("yh2", [IMGS_PER_CORE, 6, 32, 32, 2], F16,
                           kind="ExternalInput")
    yh1_d = nc.dram_tensor("yh1", [IMGS_PER_CORE, 6, 64, 64, 2], F16,
                           kind="ExternalInput")
    yh0_d = nc.dram_tensor("yh0", [IMGS_PER_CORE, 6, 128, 128, 2], F16,
                           kind="ExternalInput")
    out_d = nc.dram_tensor("out", [IMGS_PER_CORE, 256, 256], F16,
                           kind="ExternalOutput")
    mat_d = {k: nc.dram_tensor(k, list(shp), F16, kind="ExternalInput")
             for k, shp in MAT_SHAPES.items()}

    def flat2(ap):
        return ap.rearrange("a b c -> a (b c)")

    with TileContext(nc) as tc:
        with tc.tile_pool(name="mats", bufs=1) as matpool, \
             tc.tile_pool(name="data", bufs=4) as datapool, \
             tc.tile_pool(name="mid", bufs=4) as midpool, \
             tc.tile_pool(name="outp", bufs=4) as outpool, \
             tc.tile_pool(name="ps3", bufs=1, space="PSUM") as ps3pool, \
             tc.tile_pool(name="ps2c", bufs=1, space="PSUM") as ps2cpool, \
             tc.tile_pool(name="ps2r", bufs=1, space="PSUM") as ps2rpool, \
             tc.tile_pool(name="ps1c", bufs=1, space="PSUM") as ps1cpool, \
             tc.tile_pool(name="ps1r", bufs=1, space="PSUM") as ps1rpool:

            # L3/L2 matrices first so image-0 matmuls can start early;
            # spread the loads across the three DMA-capable sequencers.
            mats = {}
            _order = sorted(MAT_SHAPES, key=lambda k: k.startswith(
                ('Alo', 'L1', 'Be1', 'Bo1')))
            for i, k in enumerate(_order):
                t = matpool.tile(list(MAT_SHAPES[k]), F16, tag=f"mat_{k}")
                eng = (nc.sync, nc.scalar, nc.gpsimd)[i % 3]
                eng.dma_start(out=t[:], in_=mat_d[k][:])
                mats[k] = t

            def mm(out_ap, lhsT, rhs_name, start, stop):
                nc.tensor.matmul(out_ap, lhsT, mats[rhs_name][:],
                                 start=start, stop=stop)

            for img in range(IMGS_PER_CORE):
                # =======================================================
                # Level 3: z3 [64,64] + yh2 bands -> z2 [128,128]
                # =======================================================
                z3 = datapool.tile([64, 64], F16, tag="z3")
                nc.gpsimd.dma_start(out=z3[:], in_=yl_d[img])
                lh3 = datapool.tile([64, 64], F16, tag="lh3")
                nc.gpsimd.dma_start(out=lh3[0:32, :], in_=flat2(yh2_d[img, 0]))
                nc.gpsimd.dma_start(out=lh3[32:64, :], in_=flat2(yh2_d[img, 5]))
                q3 = datapool.tile([128, 64], F16, tag="q3")
                nc.gpsimd.dma_start(out=q3[0:32, :], in_=flat2(yh2_d[img, 2]))
                nc.gpsimd.dma_start(out=q3[32:64, :], in_=flat2(yh2_d[img, 3]))
                nc.gpsimd.dma_start(out=q3[64:96, :], in_=flat2(yh2_d[img, 1]))
                nc.gpsimd.dma_start(out=q3[96:128, :], in_=flat2(yh2_d[img, 4]))

                # p3 [128, 1024] (2 banks): y1z [64: 0-128), y1b [32: 128-384),
                #   z2 [128: 384-512), y2b [32: 512-768)
                p3 = ps3pool.tile([128, 1024], F32, tag="p3")
                mm(p3[0:64, 0:128], z3[:], 'M3_lo', True, True)
                lh3R, lh3I = lh3[:, 0::2], lh3[:, 1::2]
                mm(p3[0:32, 128:384], lh3R, 'L3_hi_R', True, False)
                mm(p3[0:32, 128:384], lh3I, 'L3_hi_I', False, True)
                q3R, q3I = q3[:, 0::2], q3[:, 1::2]
                mm(p3[0:32, 512:768], q3R, 'L3_q_R', True, False)
                mm(p3[0:32, 512:768], q3I, 'L3_q_I', False, True)

                # K-stack the row-stage pieces: rowA = [y1z; y1b_e; y1b_o],
                # rowB = [y2b_e; y2b_o] (partition shifts via SBUF-SBUF DMA)
                rowA = midpool.tile([128, 128], F16, tag="rowA3")
                nc.scalar.copy(rowA[0:64, :], p3[0:64, 0:128])
                y1b_s = midpool.tile([32, 256], F16, tag="y1b3")
                nc.vector.tensor_copy(out=y1b_s[:], in_=p3[0:32, 128:384])
                y2b_s = midpool.tile([32, 256], F16, tag="y2b3")
                nc.vector.tensor_copy(out=y2b_s[:], in_=p3[0:32, 512:768])
                nc.scalar.dma_start(out=rowA[64:96, :], in_=y1b_s[:, 0:128])
                nc.scalar.dma_start(out=rowA[96:128, :], in_=y1b_s[:, 128:256])
                rowB = midpool.tile([64, 128], F16, tag="rowB3")
                nc.vector.tensor_copy(out=rowB[0:32, :], in_=y2b_s[:, 0:128])
                nc.scalar.dma_start(out=rowB[32:64, :], in_=y2b_s[:, 128:256])

                z2_p = p3[0:128, 384:512]
                mm(z2_p, rowA[:], 'R3_A', True, False)
                mm(z2_p, rowB[:], 'R3_B', False, True)
                z2_s = midpool.tile([128, 128], F16, tag="z2")
                nc.scalar.copy(z2_s[:], z2_p)

                # =======================================================
                # Level 2: z2 [128,128] + yh1 bands -> z1 [256,256]
                # =======================================================
                lh2 = datapool.tile([128, 128], F16, tag="lh2")
                nc.scalar.dma_start(out=lh2[0:64, :], in_=flat2(yh1_d[img, 0]))
                nc.scalar.dma_start(out=lh2[64:128, :],
                                    in_=flat2(yh1_d[img, 5]))
                hl2 = datapool.tile([128, 128], F16, tag="hl2")
                nc.scalar.dma_start(out=hl2[0:64, :], in_=flat2(yh1_d[img, 2]))
                nc.scalar.dma_start(out=hl2[64:128, :],
                                    in_=flat2(yh1_d[img, 3]))
                hh2 = datapool.tile([128, 128], F16, tag="hh2")
                nc.scalar.dma_start(out=hh2[0:64, :], in_=flat2(yh1_d[img, 1]))
                nc.scalar.dma_start(out=hh2[64:128, :],
                                    in_=flat2(yh1_d[img, 4]))

                # phase A: p2 [128, 1024]: y1zT [0:256), b1 [512:1024)
                p2a = ps2cpool.tile([128, 1024], F32, tag="p2")
                mm(p2a[:, 0:256], z2_s[:], 'M2_lo', True, True)
                lh2R, lh2I = lh2[:, 0::2], lh2[:, 1::2]
                mm(p2a[0:64, 512:1024], lh2R, 'L2_hi_R', True, False)
                mm(p2a[0:64, 512:1024], lh2I, 'L2_hi_I', False, True)
                y1zT_s = midpool.tile([128, 256], F16, tag="y1zT2")
                nc.scalar.copy(y1zT_s[:], p2a[:, 0:256])
                # row2 = [b1; b2] K-stacked ([e | o] along columns)
                row2 = midpool.tile([128, 512], F16, tag="row2")
                nc.vector.tensor_copy(out=row2[0:64, :],
                                      in_=p2a[0:64, 512:1024])

                # phase B (same slot): b2 [0:512)
                p2b = ps2cpool.tile([128, 1024], F32, tag="p2")
                hl2R, hl2I = hl2[:, 0::2], hl2[:, 1::2]
                hh2R, hh2I = hh2[:, 0::2], hh2[:, 1::2]
                mm(p2b[0:64, 0:512], hl2R, 'L2_lo_R', True, False)
                mm(p2b[0:64, 0:512], hl2I, 'L2_lo_I', False, False)
                mm(p2b[0:64, 0:512], hh2R, 'L2_hi_R', False, False)
                mm(p2b[0:64, 0:512], hh2I, 'L2_hi_I', False, True)
                b2_s = midpool.tile([64, 512], F16, tag="b2_2")
                nc.vector.tensor_copy(out=b2_s[:], in_=p2b[0:64, 0:512])
                nc.scalar.dma_start(out=row2[64:128, :], in_=b2_s[:])

                # row stage -> z1 [256,256] as [128, 512] (cols 0:256 = rows
                # 0:128, cols 256:512 = rows 128:256)
                p2r = ps2rpool.tile([128, 512], F32, tag="p2r")
                z1_s = midpool.tile([128, 512], F16, tag="z1")
                for m in range(2):
                    zc = p2r[:, m * 256:(m + 1) * 256]
                    msl = slice(m * 128, (m + 1) * 128)
                    mm(zc, y1zT_s[:, msl], 'M2_lo', True, False)
                    mm(zc, row2[:, msl], 'R2_E', False, False)
                    mm(zc, row2[:, 256 + m * 128:256 + (m + 1) * 128],
                       'R2_O', False, True)
                    if m == 0:
                        nc.scalar.copy(z1_s[:, 0:256], zc)
                    else:
                        nc.vector.tensor_copy(out=z1_s[:, 256:512], in_=zc)

                # =======================================================
                # Level 1: z1 [256,256] + yh0 bands [128,128] -> out
                # =======================================================
                o_t = {}
                for o in range(6):
                    t = datapool.tile([128, 256], F16, tag=f"yh0_o{o}")
                    eng = nc.sync if o % 2 == 0 else nc.gpsimd
                    eng.dma_start(out=t[:], in_=flat2(yh0_d[img, o]))
                    o_t[o] = t

                # phase A: p1 [128, 1024]: y1zT a|b [0:512), y1b [512:1024)
                p1a = ps1cpool.tile([128, 1024], F32, tag="p1")
                for m in range(2):
                    yz = p1a[:, m * 256:(m + 1) * 256]
                    mm(yz, z1_s[:, m * 128:m * 128 + 128], 'Alo_a',
                       True, False)
                    mm(yz, z1_s[:, 256 + m * 128:256 + m * 128 + 128],
                       'Alo_b', False, True)
                y1b_p = p1a[:, 512:1024]
                mm(y1b_p, o_t[0][:, 0::2], 'L1hi_w1r', True, False)
                mm(y1b_p, o_t[5][:, 0::2], 'L1hi_w2r', False, False)
                mm(y1b_p, o_t[0][:, 1::2], 'L1hi_w1i', False, False)
                mm(y1b_p, o_t[5][:, 1::2], 'L1hi_w2i', False, True)
                y1zT1_s = midpool.tile([128, 512], F16, tag="y1zT1")
                nc.scalar.copy(y1zT1_s[:, 0:256], p1a[:, 0:256])
                nc.scalar.copy(y1zT1_s[:, 256:512], p1a[:, 256:512])
                y1b1_s = midpool.tile([128, 512], F16, tag="y1b1")
                nc.vector.tensor_copy(out=y1b1_s[:], in_=y1b_p)

                # phase B (same slot): y2b e|o [0:512)
                p1b = ps1cpool.tile([128, 1024], F32, tag="p1")
                y2b_p = p1b[:, 0:512]
                mm(y2b_p, o_t[2][:, 0::2], 'L1lo_w1r', True, False)
                mm(y2b_p, o_t[3][:, 0::2], 'L1lo_w2r', False, False)
                mm(y2b_p, o_t[2][:, 1::2], 'L1lo_w1i', False, False)
                mm(y2b_p, o_t[3][:, 1::2], 'L1lo_w2i', False, False)
                mm(y2b_p, o_t[1][:, 0::2], 'L1hi_w1r', False, False)
                mm(y2b_p, o_t[4][:, 0::2], 'L1hi_w2r', False, False)
                mm(y2b_p, o_t[1][:, 1::2], 'L1hi_w1i', False, False)
                mm(y2b_p, o_t[4][:, 1::2], 'L1hi_w2i', False, True)
                y2b1_s = midpool.tile([128, 512], F16, tag="y2b1")
                nc.vector.tensor_copy(out=y2b1_s[:], in_=y2b_p)

                # row stage -> out [256, 256] in two h-chunks
                p1r = ps1rpool.tile([128, 512], F32, tag="p1r")
                for m in range(2):
                    oc = p1r[:, m * 256:(m + 1) * 256]
                    msl = slice(m * 128, (m + 1) * 128)
                    osl = slice(256 + m * 128, 256 + (m + 1) * 128)
                    mm(oc, y1zT1_s[:, msl], 'Alo_a', True, False)
                    mm(oc, y1zT1_s[:, osl], 'Alo_b', False, False)
                    mm(oc, y1b1_s[:, msl], 'Be1_lo', False, False)
                    mm(oc, y1b1_s[:, osl], 'Bo1_lo', False, False)
                    mm(oc, y2b1_s[:, msl], 'Be1_hi', False, False)
                    mm(oc, y2b1_s[:, osl], 'Bo1_hi', False, True)
                    ot = outpool.tile([128, 256], F16, tag=f"out_m{m}")
                    if m == 0:
                        nc.scalar.copy(ot[:], oc)
                    else:
                        nc.vector.tensor_copy(out=ot[:], in_=oc)
                    (nc.sync if m == 0 else nc.gpsimd).dma_start(
                        out=out_d[img, m * 128:(m + 1) * 128, :], in_=ot[:])

    split_excess_waits(nc)
    return nc


# ---------------------------------------------------------------------------
# Entry point
# ---------------------------------------------------------------------------
_NC_CACHE = []
_LAST_RESULT = []  # last BassKernelResults (exec_time_ns when BASS_TRACE=1)


def _axon_reset():
    try:
        import ctypes
        lib = ctypes.CDLL('/opt/axon/libaxon_pjrt.so')
        lib.axon_reset.restype = ctypes.c_int64
        lib.axon_reset()
    except Exception:
        pass


def kernel(yl, yh0, yh1, yh2, g0o, g1o, g0a, g0b, g1a, g1b):
    yl = np.ascontiguousarray(np.asarray(yl, np.float16))
    yh0 = np.ascontiguousarray(np.asarray(yh0, np.float16))
    yh1 = np.ascontiguousarray(np.asarray(yh1, np.float16))
    yh2 = np.ascontiguousarray(np.asarray(yh2, np.float16))
    assert yl.shape == (8, 16, 64, 64)

    mats = build_matrices(g0o, g1o, g0a, g0b, g1a, g1b)
    if not _NC_CACHE:
        _NC_CACHE.append(build_nc())
    nc = _NC_CACHE[0]

    in_maps = []
    for core in range(N_CORES):
        m = {"yl": yl[core], "yh0": yh0[core],
             "yh1": yh1[core], "yh2": yh2[core]}
        for k in MAT_SHAPES:
            m[k] = mats[k]
        in_maps.append(m)

    try:
        res = run_bass_kernel_spmd(nc, in_maps, list(range(N_CORES)))
    except Exception as e:  # wedged exec unit: reset the axon device, retry
        if "UNAVAILABLE" not in str(e) and "unrecoverable" not in str(e):
            raise
        _axon_reset()
        res = run_bass_kernel_spmd(nc, in_maps, list(range(N_CORES)))
    _LAST_RESULT.clear()
    _LAST_RESULT.append(res)
    out = np.stack([res.results[i]["out"] for i in range(N_CORES)], axis=0)
    return np.ascontiguousarray(out.astype(np.float32))

